# revision 1
# baseline (speedup 1.0000x reference)
"""Trainium2 Bass kernel for nn_Encoder (pre-norm transformer block, LN over
sequence axis) distributed over 8 NeuronCores.

Sharding:
  - LN1/LN2 channel-sharded (C/8 = 128 channels per core, [chan, T] layout)
  - attention head-sharded (2 heads x 2 batches per core), scores computed
    transposed (S^T = k q^T) so softmax sums run through the PE via a
    ones-column appended to V, and no P transpose is needed
  - per-batch AllGather(h^T), AllGather(attn^T) in bf16 (0.5 MB/rank each),
    issued as soon as each batch is ready so they overlap compute
  - Wo column-sharded (rhs streamed from the gathered attn^T), LN2 on the
    channel slice, then AllToAll(h2^T) bf16 + AllToAll(y^T) fp32 switch from
    channel-sharded to token-sharded; FFN token-sharded with full W1/W2
  - output assembled on host from per-core token slices
"""

import numpy as np
import ml_dtypes
from contextlib import ExitStack

from concourse import bacc, bass_utils
import concourse.bass as bass
import concourse.tile as tile
import concourse.mybir as mybir
from concourse.masks import make_identity

FP32 = mybir.dt.float32
BF16 = mybir.dt.bfloat16
AF = mybir.ActivationFunctionType
ALU = mybir.AluOpType
AX = mybir.AxisListType

B, T, C, H, HS = 2, 2048, 1024, 16, 64
NCORE, P = 8, 128
TN = B * T            # 4096 flat tokens
TOK = TN // NCORE     # 512 tokens per core
F = 4 * C             # 4096
KK = C // P           # 8 k-tiles over C
EPS = 1e-5
RG = [list(range(NCORE))]

_cache = {}


def _ln_stats(nc, pool, xsrc, g_sb, be_sb, n):
    """Per-partition LN coefficients over the free axis of xsrc [P, n].
    Returns (A, Bv) with h = x*A + Bv. Unbiased var, eps outside sqrt."""
    s1 = pool.tile([P, 1], FP32, tag="s1")
    s2 = pool.tile([P, 1], FP32, tag="s2")
    scr = pool.tile([P, n], FP32, tag="scr")
    nc.vector.reduce_sum(s1[:], xsrc, axis=AX.X)
    nc.vector.scalar_tensor_tensor(
        out=scr[:], in0=xsrc, scalar=1.0, in1=xsrc,
        op0=ALU.mult, op1=ALU.mult, accum_out=s2[:])
    mean = pool.tile([P, 1], FP32, tag="mean")
    nc.vector.tensor_scalar_mul(mean[:], s1[:], 1.0 / n)
    ss = pool.tile([P, 1], FP32, tag="ss")
    nc.vector.tensor_mul(ss[:], s1[:], s1[:])
    var = pool.tile([P, 1], FP32, tag="var")
    nc.vector.scalar_tensor_tensor(
        out=var[:], in0=ss[:], scalar=-1.0 / n, in1=s2[:],
        op0=ALU.mult, op1=ALU.add)
    nc.vector.tensor_scalar_mul(var[:], var[:], 1.0 / (n - 1))
    den = pool.tile([P, 1], FP32, tag="den")
    nc.scalar.sqrt(den[:], var[:])
    nc.vector.tensor_scalar_add(den[:], den[:], EPS)
    rden = pool.tile([P, 1], FP32, tag="rden")
    nc.vector.reciprocal(rden[:], den[:])
    A = pool.tile([P, 1], FP32, tag="A")
    nc.vector.tensor_mul(A[:], g_sb, rden[:])
    mA = pool.tile([P, 1], FP32, tag="mA")
    nc.vector.tensor_scalar_mul(mA[:], mean[:], A[:])
    Bv = pool.tile([P, 1], FP32, tag="Bv")
    nc.vector.tensor_sub(Bv[:], be_sb, mA[:])
    return A, Bv


def build():
    nc = bacc.Bacc("TRN2", target_bir_lowering=False, debug=False,
                   num_devices=NCORE)

    def EIN(name, shape, dtype):
        return nc.dram_tensor(name, shape, dtype, kind="ExternalInput")

    x_c = EIN("x_c", [TN, P], FP32)        # x[:, :, ci]  (flat tokens, my chans)
    wq = EIN("wq", [P, KK, P], BF16)       # Wq cat(2 heads) tiled [p, kk, m]
    wk = EIN("wk", [P, KK, P], BF16)
    wv = EIN("wv", [P, KK, P], BF16)
    woc = EIN("woc", [P, KK, P], BF16)     # Wo[:, ci] tiled
    w1t = EIN("w1t", [F // P, P, KK, P], BF16)  # [32, p, kk, mc]
    w2t = EIN("w2t", [P, F // P, C], BF16)      # [p, q, n]
    bqc = EIN("bqc", [P, 1], FP32)
    bkc = EIN("bkc", [P, 1], FP32)
    boc = EIN("boc", [P, 1], FP32)
    b1t = EIN("b1t", [P, F // P], FP32)    # [p, m]
    b2r = EIN("b2r", [1, C], FP32)         # b2 row (added via ones-row matmul)
    g1 = EIN("g1", [P, 1], FP32)
    be1 = EIN("be1", [P, 1], FP32)
    g2 = EIN("g2", [P, 1], FP32)
    be2 = EIN("be2", [P, 1], FP32)
    out = nc.dram_tensor("out", [TOK, C], FP32, kind="ExternalOutput")

    with tile.TileContext(nc) as tc, ExitStack() as ctx:
        const = ctx.enter_context(tc.tile_pool(name="const", bufs=1))
        dram = ctx.enter_context(tc.tile_pool(name="dram", bufs=1, space="DRAM"))
        persist = ctx.enter_context(tc.tile_pool(name="acts", bufs=1))

        ident = const.tile([P, P], FP32)
        make_identity(nc, ident)
        ones1 = const.tile([1, P], FP32)
        nc.vector.memset(ones1[:], 1.0)

        def ldconst(t, shape, dt=FP32):
            s = const.tile(shape, dt, name=t.name + "_sb")
            nc.sync.dma_start(s[:], t.ap())
            return s

        bq_sb = ldconst(bqc, [P, 1])
        bk_sb = ldconst(bkc, [P, 1])
        bo_sb = ldconst(boc, [P, 1])
        b1_sb = ldconst(b1t, [P, F // P])
        b2_sb = ldconst(b2r, [1, C])
        g1_sb = ldconst(g1, [P, 1])
        be1_sb = ldconst(be1, [P, 1])
        g2_sb = ldconst(g2, [P, 1])
        be2_sb = ldconst(be2, [P, 1])
        wq_sb = ldconst(wq, [P, KK, P], BF16)
        wk_sb = ldconst(wk, [P, KK, P], BF16)
        wv_sb = ldconst(wv, [P, KK, P], BF16)
        woc_sb = ldconst(woc, [P, KK, P], BF16)

        # activations that live across phases 1-3
        xT = persist.tile([P, B, T], FP32)
        attnT_loc = persist.tile([P, TN], BF16)
        h2T_loc = persist.tile([P, TN], BF16)
        yT = persist.tile([P, B, T], FP32)
        hT_loc = persist.tile([P, B, T], BF16)
        qT_sb = persist.tile([P, B, T], BF16)
        kT_sb = persist.tile([P, B, T], BF16)
        vaug = persist.tile([P, B, 2, T // P, 65], BF16)

        # DRAM comm tiles (per-batch AGs so they overlap compute)
        agh_in = [dram.tile([P, T], BF16, name=f"agh_in{b}") for b in range(B)]
        agh_out = [dram.tile([C, T], BF16, addr_space="Shared",
                             name=f"agh_out{b}") for b in range(B)]
        aga_in = [dram.tile([P, T], BF16, name=f"aga_in{b}") for b in range(B)]
        aga_out = [dram.tile([C, T], BF16, addr_space="Shared",
                             name=f"aga_out{b}") for b in range(B)]
        a2h_in = dram.tile([NCORE, P, TOK], BF16)
        a2h_out = dram.tile([NCORE, P, TOK], BF16)
        a2y_in = dram.tile([NCORE, P, TOK], FP32)
        a2y_out = dram.tile([NCORE, P, TOK], FP32)

        # ---------------- Phase 1: transpose x slice + LN1 (per batch) ------
        with tc.tile_pool(name="ph1", bufs=4) as ph1, \
             tc.tile_pool(name="ph1p", bufs=4, space="PSUM") as ph1p, \
             tc.tile_pool(name="stats", bufs=2) as stats:
            for b in range(B):
                for tt in range(T // P):
                    xc_t = ph1.tile([P, P], FP32, tag="xc")
                    nc.sync.dma_start(
                        xc_t[:], x_c.ap()[b * T + tt * P: b * T + (tt + 1) * P, :])
                    tp = ph1p.tile([P, P], FP32, tag="tp")
                    nc.tensor.transpose(tp[:], xc_t[:], ident[:])
                    nc.vector.tensor_copy(xT[:, b, tt * P:(tt + 1) * P], tp[:])
                A, Bv = _ln_stats(nc, stats, xT[:, b, :], g1_sb[:], be1_sb[:], T)
                nc.vector.tensor_scalar(
                    out=hT_loc[:, b, :], in0=xT[:, b, :],
                    scalar1=A[:], scalar2=Bv[:], op0=ALU.mult, op1=ALU.add)
                nc.sync.dma_start(agh_in[b][:], hT_loc[:, b, :])
                nc.gpsimd.collective_compute(
                    "AllGather", ALU.bypass, replica_groups=RG,
                    ins=[agh_in[b].opt()], outs=[agh_out[b].opt()])

        # ---------------- Phase 2a: QKV ----------------
        nc.vector.memset(vaug[:, :, :, :, 64], 1.0)
        with tc.tile_pool(name="hst", bufs=1) as hst, \
             tc.tile_pool(name="qkp", bufs=4, space="PSUM") as qkp:
            hT_st = hst.tile([P, KK, B, T], BF16)   # 64KB/part, freed post-QKV
            for kk in range(KK):
                for b in range(B):
                    nc.sync.dma_start(
                        hT_st[:, kk, b, :],
                        agh_out[b].rearrange("(kk p) n -> p kk n", p=P)[:, kk, :])
            for b in range(B):
                for w_sb, bias_sb, dst in ((wq_sb, bq_sb, qT_sb),
                                           (wk_sb, bk_sb, kT_sb)):
                    for j in range(T // 512):
                        ps = qkp.tile([P, 512], FP32, tag="mm")
                        for kk in range(KK):
                            nc.tensor.matmul(
                                ps[:], lhsT=w_sb[:, kk, :],
                                rhs=hT_st[:, kk, b, j * 512:(j + 1) * 512],
                                start=(kk == 0), stop=(kk == KK - 1))
                        nc.vector.tensor_scalar_add(
                            dst[:, b, j * 512:(j + 1) * 512], ps[:], bias_sb[:])
                for tt in range(T // P):
                    vps_full = qkp.tile([P, 512], FP32, tag="mm", name="vps")
                    vps = vps_full[:, 0:P]
                    for kk in range(KK):
                        nc.tensor.matmul(
                            vps, lhsT=hT_st[:, kk, b, tt * P:(tt + 1) * P],
                            rhs=wv_sb[:, kk, :],
                            start=(kk == 0), stop=(kk == KK - 1))
                    for hd in range(2):
                        nc.vector.tensor_copy(
                            vaug[:, b, hd, tt, 0:64],
                            vps[:, hd * 64:(hd + 1) * 64])

        # ---------------- Phase 2b: attention ----------------
        with tc.tile_pool(name="ph2l", bufs=6) as ph2l, \
             tc.tile_pool(name="sp", bufs=2, space="PSUM") as sp, \
             tc.tile_pool(name="attp", bufs=3, space="PSUM") as attp:
            for b in range(B):
                for hd in range(2):
                    att_h = [attp.tile([65, T // 2], FP32, tag="att",
                                       name=f"att{b}{hd}{jh}") for jh in range(2)]
                    for k in range(T // P):
                        p_tiles = []
                        for j in range(T // 512):
                            s_ps = sp.tile([P, 512], FP32, tag="s")
                            nc.tensor.matmul(
                                s_ps[:],
                                lhsT=kT_sb[hd * 64:(hd + 1) * 64, b, k * P:(k + 1) * P],
                                rhs=qT_sb[hd * 64:(hd + 1) * 64, b, j * 512:(j + 1) * 512],
                                start=True, stop=True)
                            p_sb = ph2l.tile([P, 512], BF16, tag="p",
                                             name=f"p{j}")
                            nc.scalar.activation(p_sb[:], s_ps[:], AF.Exp,
                                                 scale=float(HS) ** -0.5)
                            p_tiles.append(p_sb)
                        for j in range(T // 512):
                            nc.tensor.matmul(
                                att_h[j // 2][:, (j % 2) * 512:(j % 2 + 1) * 512],
                                lhsT=vaug[:, b, hd, k, :], rhs=p_tiles[j][:],
                                start=(k == 0), stop=(k == T // P - 1))
                    for jh in range(2):
                        rden = ph2l.tile([1, T // 2], FP32, tag="rden")
                        nc.vector.reciprocal(rden[:], att_h[jh][64:65, :])
                        for jq in range(2):
                            rdps_f = sp.tile([P, 512], FP32, tag="s", name="rdps")
                            rdps = rdps_f[0:64, :]
                            nc.tensor.matmul(
                                rdps, lhsT=ones1[:, 0:64],
                                rhs=rden[:, jq * 512:(jq + 1) * 512],
                                start=True, stop=True)
                            rd_sb = ph2l.tile([64, 512], FP32, tag="rd_sb")
                            nc.vector.tensor_copy(rd_sb[:], rdps)
                            nc.vector.tensor_mul(
                                attnT_loc[hd * 64:(hd + 1) * 64,
                                          b * T + jh * 1024 + jq * 512:
                                          b * T + jh * 1024 + (jq + 1) * 512],
                                att_h[jh][0:64, jq * 512:(jq + 1) * 512], rd_sb[:])
                nc.sync.dma_start(aga_in[b][:], attnT_loc[:, b * T:(b + 1) * T])
                nc.gpsimd.collective_compute(
                    "AllGather", ALU.bypass, replica_groups=RG,
                    ins=[aga_in[b].opt()], outs=[aga_out[b].opt()])

        # ---------------- Phase 3: Wo col-shard (streamed rhs) + LN2 --------
        with tc.tile_pool(name="ph3", bufs=16) as ph3, \
             tc.tile_pool(name="ph3p", bufs=4, space="PSUM") as ph3p, \
             tc.tile_pool(name="stats3", bufs=2) as stats3:
            for b in range(B):
                for j in range(T // 512):
                    yps = ph3p.tile([P, 512], FP32, tag="y")
                    for kk in range(KK):
                        a_t = ph3.tile([P, 512], BF16, tag="a_t")
                        src_v = aga_out[b].rearrange("(kk p) n -> p kk n", p=P)
                        nc.sync.dma_start(
                            a_t[:, 0:256],
                            src_v[:, kk, j * 512: j * 512 + 256])
                        nc.gpsimd.dma_start(
                            a_t[:, 256:512],
                            src_v[:, kk, j * 512 + 256:(j + 1) * 512])
                        nc.tensor.matmul(
                            yps[:], lhsT=woc_sb[:, kk, :], rhs=a_t[:],
                            start=(kk == 0), stop=(kk == KK - 1))
                    nc.vector.scalar_tensor_tensor(
                        out=yT[:, b, j * 512:(j + 1) * 512], in0=yps[:],
                        scalar=bo_sb[:], in1=xT[:, b, j * 512:(j + 1) * 512],
                        op0=ALU.add, op1=ALU.add)
                A2, Bv2 = _ln_stats(nc, stats3, yT[:, b, :], g2_sb[:], be2_sb[:], T)
                nc.vector.tensor_scalar(
                    out=h2T_loc[:, b * T:(b + 1) * T], in0=yT[:, b, :],
                    scalar1=A2[:], scalar2=Bv2[:], op0=ALU.mult, op1=ALU.add)

        for j in range(NCORE):
            nc.sync.dma_start(a2h_in[j], h2T_loc[:, j * TOK:(j + 1) * TOK])
        nc.gpsimd.collective_compute(
            "AllToAll", ALU.bypass, replica_groups=RG,
            ins=[a2h_in.opt()], outs=[a2h_out.opt()])
        for j in range(NCORE):
            nc.sync.dma_start(
                a2y_in[j], yT.rearrange("p b t -> p (b t)")[:, j * TOK:(j + 1) * TOK])
        nc.gpsimd.collective_compute(
            "AllToAll", ALU.bypass, replica_groups=RG,
            ins=[a2y_in.opt()], outs=[a2y_out.opt()])

        # ---------------- Phase 4: FFN token-sharded ----------------
        with tc.tile_pool(name="ph4", bufs=1) as ph4, \
             tc.tile_pool(name="ph4l", bufs=4) as ph4l, \
             tc.tile_pool(name="ph4o", bufs=2) as ph4o:
            h2tok = ph4.tile([P, KK, TOK], BF16)
            engs = (nc.sync, nc.gpsimd, nc.sync, nc.gpsimd)
            for kk in range(KK):
                engs[kk % 4].dma_start(h2tok[:, kk, :], a2h_out[kk])
            ytok = ph4.tile([P, TOK // P, C], FP32)   # 16KB/part
            uT = ph4.tile([P, F // P, TOK], BF16)     # 32KB/part
            with tc.tile_pool(name="up", bufs=4, space="PSUM") as up:
                # y blocks: stage, PE-transpose to token-major [tok, chan]
                for kk in range(KK):
                    yb = ph4l.tile([P, TOK], FP32, tag="yb")
                    engs[kk % 4].dma_start(yb[:], a2y_out[kk])
                    for tt in range(TOK // P):
                        ytp = up.tile([P, P], FP32, tag="u", name="ytp")
                        nc.tensor.transpose(ytp[:], yb[:, tt * P:(tt + 1) * P],
                                            ident[:])
                        nc.vector.tensor_copy(ytok[:, tt, kk * P:(kk + 1) * P],
                                              ytp[:])
                for m in range(F // P):
                    w1_sl = ph4l.tile([P, KK, P], BF16, tag="w1", bufs=6)
                    nc.sync.dma_start(w1_sl[:, 0:KK // 2, :], w1t.ap()[m][:, 0:KK // 2, :])
                    nc.gpsimd.dma_start(w1_sl[:, KK // 2:KK, :], w1t.ap()[m][:, KK // 2:KK, :])
                    ups = up.tile([P, TOK], FP32, tag="u")
                    for kk in range(KK):
                        nc.tensor.matmul(
                            ups[:], lhsT=w1_sl[:, kk, :], rhs=h2tok[:, kk, :],
                            start=(kk == 0), stop=(kk == KK - 1))
                    nc.scalar.activation(uT[:, m, :], ups[:], AF.Relu,
                                         bias=b1_sb[:, m:m + 1], scale=1.0)
            with tc.tile_pool(name="zp", bufs=4, space="PSUM") as zp:
                zt = [zp.tile([P, C], FP32, tag="z", name=f"z{mm}")
                      for mm in range(TOK // P)]
                for q in range(F // P):
                    w2_sl = ph4l.tile([P, C], BF16, tag="w2", bufs=6)
                    nc.sync.dma_start(w2_sl[:, 0:512], w2t.ap()[:, q, 0:512])
                    nc.gpsimd.dma_start(w2_sl[:, 512:C], w2t.ap()[:, q, 512:C])
                    for mm in range(TOK // P):
                        for nch in range(C // 512):
                            nc.tensor.matmul(
                                zt[mm][:, nch * 512:(nch + 1) * 512],
                                lhsT=uT[:, q, mm * P:(mm + 1) * P],
                                rhs=w2_sl[:, nch * 512:(nch + 1) * 512],
                                start=(q == 0), stop=False)
                for mm in range(TOK // P):
                    for nch in range(C // 512):
                        # += b2 via ones-row product; closes the psum group
                        nc.tensor.matmul(
                            zt[mm][:, nch * 512:(nch + 1) * 512],
                            lhsT=ones1[:, 0:P],
                            rhs=b2_sb[:, nch * 512:(nch + 1) * 512],
                            start=False, stop=True)
                    o_sb = ph4o.tile([P, C], FP32, tag="o")
                    nc.vector.tensor_add(o_sb[:], zt[mm][:], ytok[:, mm, :])
                    nc.sync.dma_start(out.ap()[mm * P:(mm + 1) * P, :], o_sb[:])

    nc.compile()
    return nc

def prep_inputs(x, Wq, bq, Wk, bk, Wv, bv, Wo, bo, W1, b1, W2, b2,
                gamma1, beta1, gamma2, beta2):
    bf = ml_dtypes.bfloat16
    xf = np.asarray(x, np.float32).reshape(TN, C)
    # softmax rows sum to 1, so the v bias is equivalent to adding
    # concat_h(bv) @ Wo to the attention-projection bias
    bo_eff = (np.asarray(bo, np.float64)
              + np.asarray(bv, np.float64).reshape(C) @ np.asarray(Wo, np.float64)
              ).astype(np.float32)
    in_maps = []
    for i in range(NCORE):
        ci = slice(P * i, P * (i + 1))
        hA, hB = 2 * i, 2 * i + 1

        def tile_km(wcat):  # [C, 128] -> [p, kk, m]
            return np.ascontiguousarray(
                wcat.reshape(KK, P, P).transpose(1, 0, 2)).astype(bf)

        wq_cat = np.concatenate([Wq[hA], Wq[hB]], axis=1)
        wk_cat = np.concatenate([Wk[hA], Wk[hB]], axis=1)
        wv_cat = np.concatenate([Wv[hA], Wv[hB]], axis=1)
        in_maps.append({
            "x_c": np.ascontiguousarray(xf[:, ci]),
            "wq": tile_km(wq_cat),
            "wk": tile_km(wk_cat),
            "wv": tile_km(wv_cat),
            "woc": tile_km(np.ascontiguousarray(Wo[:, ci])),
            "w1t": np.ascontiguousarray(
                W1.reshape(KK, P, F // P, P).transpose(2, 1, 0, 3)).astype(bf),
            "w2t": np.ascontiguousarray(
                W2.reshape(F // P, P, C).transpose(1, 0, 2)).astype(bf),
            "bqc": np.concatenate([bq[hA], bq[hB]])[:, None].astype(np.float32),
            "bkc": np.concatenate([bk[hA], bk[hB]])[:, None].astype(np.float32),
            "boc": bo_eff[ci][:, None].astype(np.float32),
            "b1t": np.ascontiguousarray(
                b1.reshape(F // P, P).T).astype(np.float32),
            "b2r": b2[None, :].astype(np.float32),
            "g1": gamma1[ci][:, None].astype(np.float32),
            "be1": beta1[ci][:, None].astype(np.float32),
            "g2": gamma2[ci][:, None].astype(np.float32),
            "be2": beta2[ci][:, None].astype(np.float32),
        })
    return in_maps


def kernel(**inputs):
    inputs = {k: np.asarray(v) for k, v in inputs.items()}
    if "nc" not in _cache:
        _cache["nc"] = build()
    nc = _cache["nc"]
    in_maps = prep_inputs(**inputs)
    res = bass_utils.run_bass_kernel_spmd(nc, in_maps, core_ids=list(range(NCORE)))
    out = np.concatenate([res.results[i]["out"] for i in range(NCORE)], axis=0)
    return out.reshape(B, T, C).astype(np.float32)



# revision 28
# speedup vs baseline: 1.0868x; 1.0868x over previous
"""Trainium2 Bass kernel for nn_Encoder (pre-norm transformer block, LN over
sequence axis) distributed over 8 NeuronCores.

v2 design (vs v1 baseline at ~800us):
  - x pre-transposed on HOST -> no PE transposes / phase-1 is just DMA + LN1
  - single combined AllGather of h^T (both batches, bf16)
  - attention: scores packed 2-heads-per-PE-pass via row groups (K=64 tiles at
    rows 0-63 / 64-127), one [128,1024] Exp per (b,qchunk,k) covering both
    heads (amortizes the 352-cycle ACT overhead), softmax denom via ones-column
    in V, reciprocal_approx_fast on the [1,512] denom row, PE broadcast
  - v computed channel-major (N=512 matmuls) then flipped key-major via
    XBAR DMA transpose (no engine time)
  - PE queue kept dense across the ACT(exp)-paced attention stretch by
    interleaving QKV(b1), Wo(b0), and FFN-W1(b0-half) matmuls at query-chunk
    boundaries
  - FFN output computed channel-major (z^T = W2^T u^T) -> no output transposes;
    residual y arrives channel-major from the combined {h2|y} AllToAll (bf16)
  - per-batch AllToAll carries h2 and y together (one collective per batch)
  - final output is [C, TOK] per core; host transposes/reassembles
"""

import numpy as np
import ml_dtypes
from contextlib import ExitStack

from concourse import bacc, bass_utils
import concourse.bass as bass
import concourse.tile as tile
import concourse.mybir as mybir

FP32 = mybir.dt.float32
BF16 = mybir.dt.bfloat16
AF = mybir.ActivationFunctionType
ALU = mybir.AluOpType
AX = mybir.AxisListType

B, T, C, H, HS = 2, 2048, 1024, 16, 64
NCORE, P = 8, 128
TN = B * T            # 4096 flat tokens
TOK = TN // NCORE     # 512 tokens per core (256 from each batch)
HTOK = TOK // 2       # 256 tokens per batch per core
F = 4 * C             # 4096
M1 = F // P           # 32 f-blocks
KK = C // P           # 8 k-tiles over C
EPS = 1e-5
RG = [list(range(NCORE))]

_cache = {}


def _ln_stats(nc, pool, xsrc, g_sb, be_sb, n, scr):
    """Per-partition LN coefficients over the free axis of xsrc [P, n].
    Returns (A, Bv) with h = x*A + Bv. Unbiased var, eps outside sqrt.
    `scr` is a dead [P, n] AP that absorbs the squared values (only the
    fp32 accumulator output matters)."""
    s1 = pool.tile([P, 1], FP32, tag="s1")
    s2 = pool.tile([P, 1], FP32, tag="s2")
    nc.vector.reduce_sum(s1[:], xsrc, axis=AX.X)
    nc.vector.scalar_tensor_tensor(
        out=scr, in0=xsrc, scalar=1.0, in1=xsrc,
        op0=ALU.mult, op1=ALU.mult, accum_out=s2[:])
    mean = pool.tile([P, 1], FP32, tag="mean")
    nc.vector.tensor_scalar_mul(mean[:], s1[:], 1.0 / n)
    ss = pool.tile([P, 1], FP32, tag="ss")
    nc.vector.tensor_mul(ss[:], s1[:], s1[:])
    var = pool.tile([P, 1], FP32, tag="var")
    nc.vector.scalar_tensor_tensor(
        out=var[:], in0=ss[:], scalar=-1.0 / n, in1=s2[:],
        op0=ALU.mult, op1=ALU.add)
    nc.vector.tensor_scalar_mul(var[:], var[:], 1.0 / (n - 1))
    den = pool.tile([P, 1], FP32, tag="den")
    nc.scalar.sqrt(den[:], var[:])
    nc.vector.tensor_scalar_add(den[:], den[:], EPS)
    rden = pool.tile([P, 1], FP32, tag="rden")
    nc.vector.reciprocal(rden[:], den[:])
    A = pool.tile([P, 1], FP32, tag="A")
    nc.vector.tensor_mul(A[:], g_sb, rden[:])
    mA = pool.tile([P, 1], FP32, tag="mA")
    nc.vector.tensor_scalar_mul(mA[:], mean[:], A[:])
    Bv = pool.tile([P, 1], FP32, tag="Bv")
    nc.vector.tensor_sub(Bv[:], be_sb, mA[:])
    return A, Bv


def build(dbg=False):
    nc = bacc.Bacc("TRN2", target_bir_lowering=False, debug=False,
                   num_devices=NCORE)

    def EIN(name, shape, dtype):
        return nc.dram_tensor(name, shape, dtype, kind="ExternalInput")

    xt = EIN("xt", [P, TN], FP32)          # x^T slice [my chans, flat tokens]
    wq = EIN("wq", [P, KK, P], BF16)       # Wq cat(2 heads) tiled [p, kk, m]
    wk = EIN("wk", [P, KK, P], BF16)
    wv = EIN("wv", [P, KK, P], BF16)
    woc = EIN("woc", [P, KK, P], BF16)     # Wo[:, ci] tiled
    w1t = EIN("w1t", [M1, P, KK, P], BF16)  # [32, c-part, kk, f-col]
    w2n = EIN("w2n", [P, M1, KK, P], BF16)  # [f-part, q, c-chunk, c-col]
    bqc = EIN("bqc", [P, 1], FP32)
    bkc = EIN("bkc", [P, 1], FP32)
    boc = EIN("boc", [P, 1], FP32)
    b1t = EIN("b1t", [P, M1], FP32)        # [f-part, m]
    b2c = EIN("b2c", [P, KK], FP32)        # [c-col, c-chunk]
    g1 = EIN("g1", [P, 1], FP32)
    be1 = EIN("be1", [P, 1], FP32)
    g2 = EIN("g2", [P, 1], FP32)
    be2 = EIN("be2", [P, 1], FP32)
    out = nc.dram_tensor("out", [C, TOK], FP32, kind="ExternalOutput")
    if dbg:
        dq = nc.dram_tensor("dq", [P, TN], BF16, kind="ExternalOutput")
        dk = nc.dram_tensor("dk", [P, TN], BF16, kind="ExternalOutput")
        dv = nc.dram_tensor("dv", [P, B * 2 * (T // P) * 65], BF16,
                            kind="ExternalOutput")
        da = nc.dram_tensor("da", [P, TN], BF16, kind="ExternalOutput")
        dy = nc.dram_tensor("dy", [P, TN], FP32, kind="ExternalOutput")
        dh2 = nc.dram_tensor("dh2", [P, TN], BF16, kind="ExternalOutput")
        dht = nc.dram_tensor("dht", [P, B, KK, HTOK], BF16,
                             kind="ExternalOutput")
        ds = nc.dram_tensor("ds", [P, 1024], FP32, kind="ExternalOutput")
        dp = nc.dram_tensor("dp", [P, 1024], BF16, kind="ExternalOutput")
        datt = nc.dram_tensor("datt", [P, 2, 512], FP32, kind="ExternalOutput")
        drd = nc.dram_tensor("drd", [P, 512], FP32, kind="ExternalOutput")
        du = nc.dram_tensor("du", [P, M1 * TOK], BF16, kind="ExternalOutput")

    with tile.TileContext(nc) as tc, ExitStack() as ctx:
        const = ctx.enter_context(tc.tile_pool(name="const", bufs=1))
        dram = ctx.enter_context(tc.tile_pool(name="dram", bufs=1, space="DRAM"))
        persist = ctx.enter_context(tc.tile_pool(name="acts", bufs=1))
        stats = ctx.enter_context(tc.tile_pool(name="stats", bufs=1))

        ones1 = const.tile([1, P], FP32)
        nc.vector.memset(ones1[:], 1.0)

        def ldconst(t, shape, dt=FP32):
            s = const.tile(shape, dt, name=t.name + "_sb")
            nc.sync.dma_start(s[:], t.ap())
            return s

        bq_sb = ldconst(bqc, [P, 1])
        bk_sb = ldconst(bkc, [P, 1])
        bo_sb = ldconst(boc, [P, 1])
        b1_sb = ldconst(b1t, [P, M1])
        b2_sb = ldconst(b2c, [P, KK])
        g1_sb = ldconst(g1, [P, 1])
        be1_sb = ldconst(be1, [P, 1])
        g2_sb = ldconst(g2, [P, 1])
        be2_sb = ldconst(be2, [P, 1])
        wq_sb = ldconst(wq, [P, KK, P], BF16)
        wk_sb = ldconst(wk, [P, KK, P], BF16)
        wv_sb = ldconst(wv, [P, KK, P], BF16)
        woc_sb = ldconst(woc, [P, KK, P], BF16)

        # activations with long lifetimes
        ffs = ctx.enter_context(tc.tile_pool(name="ffs", bufs=1))
        uT = ffs.tile([P, M1, TOK], BF16)
        h2tok = [ffs.tile([P, KK, HTOK], BF16, name=f"h2tok{b}")
                 for b in range(B)]
        ystage = [ffs.tile([P, KK, HTOK], BF16, name=f"ystage{b}")
                  for b in range(B)]
        xT = persist.tile([P, B, T], FP32)
        qT = persist.tile([P, B, T], BF16)
        kT = persist.tile([P, B, T], BF16)
        vaug = persist.tile([P, B, 2, T // P, 65], BF16)
        attnT = persist.tile([P, B, T], BF16)   # also reused for bf16 y copy
        yT = persist.tile([P, B, T], FP32)
        h2T = persist.tile([P, B, T], BF16)

        # DRAM comm tiles
        agh_in = dram.tile([P, TN], BF16)
        agh_out = dram.tile([C, TN], BF16, addr_space="Shared")
        aga_in = [dram.tile([P, T], BF16, name=f"aga_in{b}") for b in range(B)]
        aga_out = [dram.tile([C, T], BF16, addr_space="Shared",
                             name=f"aga_out{b}") for b in range(B)]
        a2_in = [dram.tile([NCORE, P, TOK], BF16, name=f"a2_in{b}")
                 for b in range(B)]
        a2_out = [dram.tile([NCORE, P, TOK], BF16, name=f"a2_out{b}")
                  for b in range(B)]

        nc.vector.memset(vaug[:, :, :, :, 64], 1.0)

        # ---------------- Phase A: x load + LN1 + combined AllGather -------
        xT_f = xT.rearrange("p b t -> p (b t)")
        for q4 in range(4):
            eng = (nc.sync, nc.gpsimd, nc.scalar, nc.sync)[q4]
            sl = slice(q4 * (TN // 4), (q4 + 1) * (TN // 4))
            eng.dma_start(xT_f[:, sl], xt.ap()[:, sl])
        with tc.tile_pool(name="ph1", bufs=2) as ph1:
            for b in range(B):
                A, Bv = _ln_stats(nc, stats, xT[:, b, :], g1_sb[:], be1_sb[:],
                                  T, scr=attnT[:, b, :])
                hloc = ph1.tile([P, T], BF16, tag="hloc")
                nc.vector.tensor_scalar(
                    out=hloc[:], in0=xT[:, b, :],
                    scalar1=A[:], scalar2=Bv[:], op0=ALU.mult, op1=ALU.add)
                nc.sync.dma_start(agh_in[:, b * T:(b + 1) * T], hloc[:])
            nc.gpsimd.collective_compute(
                "AllGather", ALU.bypass, replica_groups=RG,
                ins=[agh_in.opt()], outs=[agh_out.opt()])

        agh_v = agh_out.rearrange("(kk p) n -> p kk n", p=P)

        with ExitStack() as phctx:
            hstp = phctx.enter_context(tc.tile_pool(name="hst", bufs=1))
            vtp = phctx.enter_context(tc.tile_pool(name="vt", bufs=1))

            hst = {}
            vt = {}

            def stage_h(b):
                hst[b] = hstp.tile([P, KK, T], BF16, tag="hst",
                                   name=f"hst{b}")
                for kk in range(KK):
                    eng = (nc.sync, nc.gpsimd)[kk % 2]
                    eng.dma_start(hst[b][:, kk, :],
                                  agh_v[:, kk, b * T:(b + 1) * T])

            def qkv_part(b, part, psum_pool):
                """part 0: q, 1: k, 2: v matmuls+copy, 3: v transposes."""
                if part in (0, 1):
                    w_sb, bias_sb, dst = ((wq_sb, bq_sb, qT), (wk_sb, bk_sb, kT))[part]
                    for j in range(4):
                        ps_f = psum_pool.tile([P, 512], FP32, tag="il")
                        for kk in range(KK):
                            nc.tensor.matmul(
                                ps_f[:], lhsT=w_sb[:, kk, :],
                                rhs=hst[b][:, kk, j * 512:(j + 1) * 512],
                                start=(kk == 0), stop=(kk == KK - 1))
                        nc.vector.tensor_scalar_add(
                            dst[:, b, j * 512:(j + 1) * 512], ps_f[:], bias_sb[:])
                elif part == 2:
                    vt[b] = vtp.tile([P, T], BF16, tag="vt", name=f"vt{b}")
                    for j in range(4):
                        ps_f = psum_pool.tile([P, 512], FP32, tag="il")
                        for kk in range(KK):
                            nc.tensor.matmul(
                                ps_f[:], lhsT=wv_sb[:, kk, :],
                                rhs=hst[b][:, kk, j * 512:(j + 1) * 512],
                                start=(kk == 0), stop=(kk == KK - 1))
                        nc.vector.tensor_copy(
                            vt[b][:, j * 512:(j + 1) * 512], ps_f[:])
                else:
                    # XBAR transpose needs a dense dest; stage then one
                    # strided DVE copy into the 65-wide vaug rows
                    for hd in range(2):
                        vst = vtp.tile([P, T // P, 64], BF16, tag="vst",
                                       name=f"vst{b}{hd}", bufs=2)
                        for tt in range(T // P):
                            nc.sync.dma_start(
                                vst[:, tt, :],
                                vt[b][hd * 64:(hd + 1) * 64, tt * P:(tt + 1) * P],
                                transpose=True)
                        nc.vector.tensor_copy(vaug[:, b, hd, :, 0:64], vst[:])

            # ---------------- QKV b0 ----------------
            stage_h(0)
            with tc.tile_pool(name="qkp0", bufs=2, space="PSUM") as qkp0:
                for part in range(4):
                    qkv_part(0, part, qkp0)
            stage_h(1)  # hits DMA queues during attention b0

            # ---------------- attention pools -------------
            sp = phctx.enter_context(
                tc.tile_pool(name="sp", bufs=2, space="PSUM"))
            attp = phctx.enter_context(
                tc.tile_pool(name="attp", bufs=1, space="PSUM"))
            bcp = phctx.enter_context(
                tc.tile_pool(name="bcp", bufs=1, space="PSUM"))
            ilp = phctx.enter_context(
                tc.tile_pool(name="ilp", bufs=1, space="PSUM"))
            pp = phctx.enter_context(tc.tile_pool(name="pp", bufs=2))
            amisc = phctx.enter_context(tc.tile_pool(name="amisc", bufs=1))

            def attn_qchunk(b, qc):
                """One 512-query chunk of attention for both heads of batch b."""
                att = [attp.tile([65, 512], FP32, tag=f"att{hd}",
                                 name=f"att{b}{qc}{hd}") for hd in range(2)]
                qsl = slice(qc * 512, (qc + 1) * 512)
                for k in range(T // P):
                    ksl = slice(k * P, (k + 1) * P)
                    S = sp.tile([P, 1024], FP32, tag="s")
                    nc.tensor.matmul(S[:, 0:512], lhsT=kT[0:64, b, ksl],
                                     rhs=qT[0:64, b, qsl],
                                     start=True, stop=True)
                    nc.tensor.matmul(S[:, 512:1024], lhsT=kT[64:128, b, ksl],
                                     rhs=qT[64:128, b, qsl],
                                     start=True, stop=True)
                    p = pp.tile([P, 1024], BF16, tag="p")
                    nc.scalar.activation(p[:], S[:], AF.Exp,
                                         scale=float(HS) ** -0.5)
                    if dbg and b == 0 and qc == 0 and k == 0:
                        dsb = pp.tile([P, 1024], FP32, tag="dsb", name="dsb")
                        nc.vector.tensor_copy(dsb[:], S[:])
                        nc.scalar.dma_start(ds.ap(), dsb[:])
                        nc.scalar.dma_start(dp.ap(), p[:])
                    for hd in range(2):
                        nc.tensor.matmul(
                            att[hd][:], lhsT=vaug[:, b, hd, k, :],
                            rhs=p[:, hd * 512:(hd + 1) * 512],
                            start=(k == 0), stop=(k == T // P - 1))
                if dbg and b == 0 and qc == 0:
                    for hd in range(2):
                        dab = pp.tile([P, 512], FP32, tag="dab",
                                      name=f"dab{hd}")
                        nc.vector.tensor_copy(dab[0:65, :], att[hd][:])
                        nc.scalar.dma_start(datt.ap()[:, hd, :], dab[:])
                for hd in range(2):
                    den_sb = amisc.tile([1, 512], FP32, tag="den")
                    nc.vector.tensor_copy(den_sb[:], att[hd][64:65, :])
                    rden = amisc.tile([1, 512], FP32, tag="rden")
                    nc.vector.reciprocal_approx_fast(rden[:], den_sb[:])
                    dps_f = bcp.tile([64, 512], FP32, tag="bc")
                    nc.tensor.matmul(dps_f[:], lhsT=ones1[:, 0:64],
                                     rhs=rden[:], start=True, stop=True)
                    rd_sb = amisc.tile([64, 512], FP32, tag="rd")
                    nc.vector.tensor_copy(rd_sb[:], dps_f[:])
                    if dbg and b == 0 and qc == 0 and hd == 0:
                        nc.scalar.dma_start(drd.ap()[0:64, :], rd_sb[:])
                    nc.vector.tensor_mul(
                        attnT[hd * 64:(hd + 1) * 64, b, qsl],
                        att[hd][0:64, :], rd_sb[:])

            # FFN weight-streaming pools
            w1s = phctx.enter_context(tc.tile_pool(name="w1s", bufs=4))
            wos = phctx.enter_context(tc.tile_pool(name="wos", bufs=4))

            def wo_chunk(b, j):
                """y^T[:, j*512:(j+1)*512] = Wo_col @ attn_gathered + bo + x."""
                jsl = slice(j * 512, (j + 1) * 512)
                yps = ilp.tile([P, 512], FP32, tag="il", name=f"yps{b}{j}")
                aga_v = aga_out[b].rearrange("(kk p) n -> p kk n", p=P)
                for kk in range(KK):
                    a_t = wos.tile([P, 512], BF16, tag="a_t")
                    eng = (nc.sync, nc.gpsimd)[kk % 2]
                    eng.dma_start(a_t[:], aga_v[:, kk, jsl])
                    nc.tensor.matmul(yps[:], lhsT=woc_sb[:, kk, :], rhs=a_t[:],
                                     start=(kk == 0), stop=(kk == KK - 1))
                nc.vector.scalar_tensor_tensor(
                    out=yT[:, b, jsl], in0=yps[:], scalar=bo_sb[:],
                    in1=xT[:, b, jsl], op0=ALU.add, op1=ALU.add)

            def ln2_a2a(b):
                A2, Bv2 = _ln_stats(nc, stats, yT[:, b, :], g2_sb[:],
                                    be2_sb[:], T, scr=qT[:, b, :])
                nc.vector.tensor_scalar(
                    out=h2T[:, b, :], in0=yT[:, b, :],
                    scalar1=A2[:], scalar2=Bv2[:], op0=ALU.mult, op1=ALU.add)
                # bf16 copy of y for the residual A2A; attnT[:, b] is dead now
                yb16 = attnT[:, b, :]
                nc.vector.tensor_copy(yb16, yT[:, b, :])
                for j in range(NCORE):
                    tsl = slice(j * HTOK, (j + 1) * HTOK)
                    nc.gpsimd.dma_start(a2_in[b][j][:, 0:HTOK], h2T[:, b, tsl])
                    nc.gpsimd.dma_start(a2_in[b][j][:, HTOK:TOK], yb16[:, tsl])
                nc.gpsimd.collective_compute(
                    "AllToAll", ALU.bypass, replica_groups=RG,
                    ins=[a2_in[b].opt()], outs=[a2_out[b].opt()])
                for kk in range(KK):
                    nc.sync.dma_start(h2tok[b][:, kk, :],
                                      a2_out[b][kk][:, 0:HTOK])
                    nc.sync.dma_start(ystage[b][:, kk, :],
                                      a2_out[b][kk][:, HTOK:TOK])

            def w1_part(b, m0, m1):
                """u^T[f-block m, b-half tokens] for m in [m0, m1)."""
                for m in range(m0, m1):
                    w1_sl = w1s.tile([P, KK, P], BF16, tag="w1")
                    nc.sync.dma_start(w1_sl[:, 0:KK // 2, :],
                                      w1t.ap()[m][:, 0:KK // 2, :])
                    nc.gpsimd.dma_start(w1_sl[:, KK // 2:KK, :],
                                        w1t.ap()[m][:, KK // 2:KK, :])
                    ups_f = ilp.tile([P, 512], FP32, tag="il", name=f"ups{b}{m}")
                    ups = ups_f[:, 0:HTOK]
                    for kk in range(KK):
                        nc.tensor.matmul(ups, lhsT=w1_sl[:, kk, :],
                                         rhs=h2tok[b][:, kk, :],
                                         start=(kk == 0), stop=(kk == KK - 1))
                    nc.vector.tensor_scalar(
                        out=uT[:, m, b * HTOK:(b + 1) * HTOK], in0=ups,
                        scalar1=b1_sb[:, m:m + 1], scalar2=0.0,
                        op0=ALU.add, op1=ALU.max)

            # attention b0, interleaving QKV b1 at chunk boundaries
            for qc in range(4):
                attn_qchunk(0, qc)
                qkv_part(1, qc, ilp)
            if dbg:
                nc.scalar.dma_start(dq.ap(), qT.rearrange("p b t -> p (b t)"))
                nc.scalar.dma_start(dk.ap(), kT.rearrange("p b t -> p (b t)"))
                nc.scalar.dma_start(
                    dv.ap(), vaug.rearrange("p b h t e -> p (b h t e)"))
            nc.gpsimd.dma_start(aga_in[0][:], attnT[:, 0, :])
            nc.gpsimd.collective_compute(
                "AllGather", ALU.bypass, replica_groups=RG,
                ins=[aga_in[0].opt()], outs=[aga_out[0].opt()])
            if dbg:
                nc.scalar.dma_start(da.ap()[:, 0:T], attnT[:, 0, :])

            # attention b1, interleaving Wo(b0) and W1(b0 half)
            attn_qchunk(1, 0)
            wo_chunk(0, 0)
            wo_chunk(0, 1)
            attn_qchunk(1, 1)
            wo_chunk(0, 2)
            wo_chunk(0, 3)
            ln2_a2a(0)
            attn_qchunk(1, 2)
            w1_part(0, 0, 8)
            attn_qchunk(1, 3)
            if dbg:
                nc.scalar.dma_start(da.ap()[:, T:TN], attnT[:, 1, :])
            nc.gpsimd.dma_start(aga_in[1][:], attnT[:, 1, :])
            nc.gpsimd.collective_compute(
                "AllGather", ALU.bypass, replica_groups=RG,
                ins=[aga_in[1].opt()], outs=[aga_out[1].opt()])
            w1_part(0, 8, M1)

            # ---------------- tail: Wo b1, LN2 b1, A2A b1, W1 b1 ----------
            for j in range(4):
                wo_chunk(1, j)
            ln2_a2a(1)
            w1_part(1, 0, M1)
            if dbg:
                nc.scalar.dma_start(dy.ap(), yT.rearrange("p b t -> p (b t)"))
                nc.scalar.dma_start(dh2.ap(), h2T.rearrange("p b t -> p (b t)"))
                for b in range(B):
                    nc.scalar.dma_start(dht.ap()[:, b], h2tok[b][:])
                nc.scalar.dma_start(du.ap(), uT.rearrange("p m t -> p (m t)"))

        # ---------------- W2: z^T = W2^T u^T + b2 + y^T ----------------
        with tc.tile_pool(name="w2s", bufs=2) as w2s, \
             tc.tile_pool(name="zp", bufs=2, space="PSUM") as zp, \
             tc.tile_pool(name="fo", bufs=2) as fo:
                for c in range(KK):
                    w2_st = w2s.tile([P, M1, P], BF16, tag="w2c")
                    for qh, eng in ((0, nc.sync), (1, nc.gpsimd)):
                        eng.dma_start(w2_st[:, qh * 16:(qh + 1) * 16, :],
                                      w2n.ap()[:, qh * 16:(qh + 1) * 16, c, :])
                    zps = zp.tile([P, TOK], FP32, tag="z")
                    for q in range(M1):
                        nc.tensor.matmul(zps[:], lhsT=w2_st[:, q, :],
                                         rhs=uT[:, q, :],
                                         start=(q == 0), stop=(q == M1 - 1))
                    o_sb = fo.tile([P, TOK], FP32, tag="o")
                    for b in range(B):
                        hsl = slice(b * HTOK, (b + 1) * HTOK)
                        nc.vector.scalar_tensor_tensor(
                            out=o_sb[:, hsl], in0=zps[:, hsl],
                            scalar=b2_sb[:, c:c + 1], in1=ystage[b][:, c, :],
                            op0=ALU.add, op1=ALU.add)
                    nc.scalar.dma_start(out.ap()[c * P:(c + 1) * P, :], o_sb[:])

    nc.compile()
    return nc


def prep_inputs(x, Wq, bq, Wk, bk, Wv, bv, Wo, bo, W1, b1, W2, b2,
                gamma1, beta1, gamma2, beta2):
    bf = ml_dtypes.bfloat16
    xf = np.asarray(x, np.float32).reshape(TN, C)
    xfT = np.ascontiguousarray(xf.T)
    # softmax rows sum to 1, so the v bias is equivalent to adding
    # concat_h(bv) @ Wo to the attention-projection bias
    bo_eff = (np.asarray(bo, np.float64)
              + np.asarray(bv, np.float64).reshape(C) @ np.asarray(Wo, np.float64)
              ).astype(np.float32)
    w1_tiled = np.ascontiguousarray(
        W1.reshape(KK, P, M1, P).transpose(2, 1, 0, 3)).astype(bf)
    w2_tiled = np.ascontiguousarray(
        W2.reshape(M1, P, KK, P).transpose(1, 0, 2, 3)).astype(bf)
    b1_tiled = np.ascontiguousarray(b1.reshape(M1, P).T).astype(np.float32)
    b2_tiled = np.ascontiguousarray(b2.reshape(KK, P).T).astype(np.float32)
    in_maps = []
    for i in range(NCORE):
        ci = slice(P * i, P * (i + 1))
        hA, hB = 2 * i, 2 * i + 1

        def tile_km(wcat):  # [C, 128] -> [p, kk, m]
            return np.ascontiguousarray(
                wcat.reshape(KK, P, P).transpose(1, 0, 2)).astype(bf)

        wq_cat = np.concatenate([Wq[hA], Wq[hB]], axis=1)
        wk_cat = np.concatenate([Wk[hA], Wk[hB]], axis=1)
        wv_cat = np.concatenate([Wv[hA], Wv[hB]], axis=1)
        in_maps.append({
            "xt": np.ascontiguousarray(xfT[ci]),
            "wq": tile_km(wq_cat),
            "wk": tile_km(wk_cat),
            "wv": tile_km(wv_cat),
            "woc": tile_km(np.ascontiguousarray(Wo[:, ci])),
            "w1t": w1_tiled,
            "w2n": w2_tiled,
            "bqc": np.concatenate([bq[hA], bq[hB]])[:, None].astype(np.float32),
            "bkc": np.concatenate([bk[hA], bk[hB]])[:, None].astype(np.float32),
            "boc": bo_eff[ci][:, None].astype(np.float32),
            "b1t": b1_tiled,
            "b2c": b2_tiled,
            "g1": gamma1[ci][:, None].astype(np.float32),
            "be1": beta1[ci][:, None].astype(np.float32),
            "g2": gamma2[ci][:, None].astype(np.float32),
            "be2": beta2[ci][:, None].astype(np.float32),
        })
    return in_maps


def assemble_out(results):
    full = np.empty((C, TN), np.float32)
    for i in range(NCORE):
        full[:, i * HTOK:(i + 1) * HTOK] = results[i][:, 0:HTOK]
        full[:, T + i * HTOK:T + (i + 1) * HTOK] = results[i][:, HTOK:TOK]
    return np.ascontiguousarray(full.T).reshape(B, T, C)


def kernel(**inputs):
    inputs = {k: np.asarray(v) for k, v in inputs.items()}
    if "nc" not in _cache:
        _cache["nc"] = build()
    nc = _cache["nc"]
    in_maps = prep_inputs(**inputs)
    res = bass_utils.run_bass_kernel_spmd(nc, in_maps, core_ids=list(range(NCORE)))
    return assemble_out([res.results[i]["out"] for i in range(NCORE)])
